# revision 17
# baseline (speedup 1.0000x reference)
"""Trainium2 Bass kernel: segment_sum of edge features into nodes (GNN
aggregation).

out[n, :] = sum of edges[e, :] over edges with receivers[e] == n, for
n in [0, 100000), edges [1000000, 64] fp32 — distributed over 8 NeuronCores.
Cores are value-sharded by receiver range (12500 nodes each, disjoint), so no
cross-core reduction is needed; the host concatenates the shards.

Device algorithm ("block-ones matmul tower fold", fp16 end to end):
  - Edge features ride as plain fp16 (end-to-end error ~5e-4 relative vs the
    2e-2 gate), halving input traffic vs an fp32-exact hi+lo split.
  - Host splits nodes with degree > 16 into pseudo-nodes of <= 16 edges,
    sorts pseudo-nodes by degree (desc), and packs 64 per block, 2 slots per
    node per chunk: pseudo-node j of block b puts its e-th edge row at
    tokens[2j + (e&1), c0[b] + (e>>1), :].  A block occupies
    K_b = ceil(max-degree-in-block / 2) <= 8 consecutive chunks ("towers");
    padding is ~3% (odd-degree slots + within-block degree spread).
  - ONE matmul per block folds the whole tower: lhsT = static block-ones
    [128, 64] (ones2[s, m] = 1 iff s//2 == m, so out row m sums slots 2m and
    2m+1), rhs = tok[:, c:c+K, :], and the out access pattern
    [[64 part], [0, K], [1, 64]] revisits the same 64 PSUM columns for every
    chunk — PSUM's per-element has_written accumulate sums the K chunks in
    hardware.  The 64-wide lhsT halves the per-matmul LDWEIGHTS cost (53 ns)
    vs a 128-wide identity, keeping the PE comfortably under the DMA stream.
  - Matmul out free iteration is ISA-capped at 512 elements, hence K <= 8 per
    instruction — guaranteed here since pseudo-degree <= 16.
  - Two blocks stack per 128-partition group (tile_position column tiling at
    partition 0/64); 16 blocks fill one 2KB PSUM bank; one ScalarE/VectorE
    copy (alternating) casts the bank to fp16 in SBUF.  Inputs stream on the
    Sync HWDGE ring in ~2 MB slabs; outputs ride the Scalar ring.  Output is
    exactly one 64-col fp16 row per pseudo-node (~1.7 MB/core).
  - Host adds pseudo-node rows back into node rows (np.add.at over ~13k rows)
    in f32.
  - Block heights K_b are measured from the actual data (elementwise max
    across the 8 cores' sorted degree profiles) and baked into the compiled
    program inside kernel(); all cores share one SPMD schedule.
"""

import os

import numpy as np

N_EDGES = 1_000_000
N_NODES = 100_000
N_FEAT = 64
N_CORES = 8
NPC = N_NODES // N_CORES  # 12500 nodes per core
K_CAP = 16  # max edges per pseudo-node -> tower height ceil(16/2) = 8 chunks
BLK = 64  # pseudo-nodes per block (two slots each)
BPB = 16  # blocks per PSUM bank (2 partition groups x 8 column slices)
CHUNK_BUDGET = 128  # chunks per input DMA slab (16 KB/partition, ~2.1 MB)

_NC_CACHE = {}
LAST_RESULT = None


def _excl_cumsum(a):
    s = np.zeros_like(a)
    np.cumsum(a[:-1], out=s[1:])
    return s


def _input_groups(k_sched):
    """One input DMA slab per PSUM bank group (16 blocks): the slab a bank
    group consumes is exactly one DMA, so the pipeline couples tightly and
    the schedule's sorted-descending heights give a naturally tapering tail."""
    nb = len(k_sched)
    return [[b, min(nb, b + BPB)] for b in range(0, nb, BPB)]


def _build_nc(k_sched):
    """Compile the SPMD program for a static tuple of block heights."""
    if k_sched in _NC_CACHE:
        return _NC_CACHE[k_sched]

    import concourse.bass as bass
    import concourse.tile as tile
    from concourse import bacc, mybir

    F16 = mybir.dt.float16
    F32 = mybir.dt.float32

    nb = len(k_sched)
    nbd = (nb + 1) // 2  # dram col-blocks (2 blocks stack per 128 partitions)
    c0 = np.concatenate([[0], np.cumsum(k_sched)]).astype(np.int64)
    c_total = int(c0[-1])
    igroups = _input_groups(k_sched)
    ng = len(igroups)
    gmax = max(int(c0[b1] - c0[b0]) for b0, b1 in igroups)

    nc = bacc.Bacc("TRN2", target_bir_lowering=False)
    tokens = nc.dram_tensor("tokens", [128, c_total, 64], F16, kind="ExternalInput")
    ones2 = nc.dram_tensor("ones2", [128, 64], F16, kind="ExternalInput")
    out = nc.dram_tensor("out", [128, nbd, 64], F16, kind="ExternalOutput")

    with tile.TileContext(nc) as tc:
        with (
            nc.allow_low_precision(reason="fp16 staging is intentional"),
            tc.tile_pool(name="const", bufs=1) as const,
            tc.tile_pool(name="tok", bufs=4) as tokp,
            tc.tile_pool(name="ps", bufs=4, space="PSUM") as psp,
            tc.tile_pool(name="stage", bufs=3) as stp,
        ):
            ones2_t = const.tile([128, 64], F16)
            nc.scalar.dma_start(ones2_t[:], ones2[:])

            def emit_slab(g, engine):
                b0, b1 = igroups[g]
                gc0, gcn = int(c0[b0]), int(c0[b1] - c0[b0])
                t = tokp.tile([128, gmax, 64], F16, tag="tok", name="tok")
                engine.dma_start(t[:, 0:gcn, :], tokens[:, gc0 : gc0 + gcn, :])
                return t, gc0

            slabs = {0: emit_slab(0, nc.sync)}
            if ng > 1:
                slabs[1] = emit_slab(1, nc.sync)

            for g in range(ng):
                if g + 2 < ng:
                    slabs[g + 2] = emit_slab(g + 2, nc.sync)
                tok, tok_c0 = slabs.pop(g)
                b0, b1 = igroups[g]
                ps = psp.tile([128, 512], F32, tag="ps")
                for b in range(b0, b1):
                    w = b - b0
                    k = k_sched[b]
                    assert 0 < k <= 8
                    cs = int(c0[b]) - tok_c0
                    prow = 64 * (w % 2)
                    slot = w // 2
                    pslice = ps[prow : prow + 64, slot * 64 : (slot + 1) * 64]
                    o = bass.AP(
                        pslice.tensor,
                        pslice.offset,
                        [list(pslice.ap[0]), [0, k], [1, 64]],
                    )
                    nc.tensor.matmul(
                        out=o,
                        lhsT=ones2_t[:],
                        rhs=tok[:, cs : cs + k, :],
                        start=True,
                        stop=True,
                    )
                ncols = ((b1 - 1 - b0) // 2 + 1) * 64
                stage = stp.tile([128, 512], F16, tag="stage")
                # All stage copies ride the otherwise-idle VectorE: any
                # InstActivation on ScalarE triggers a ~1.3us ACT_TABLE_LOAD
                # that gates the program's opening barrier.
                nc.vector.tensor_copy(stage[:, 0:ncols], ps[:, 0:ncols])
                nc.scalar.dma_start(
                    out[:, b0 // 2 : b0 // 2 + ncols // 64, :], stage[:, 0:ncols]
                )
    nc.compile()
    _NC_CACHE[k_sched] = nc
    return nc


def _numpy_segment_sum(edges, receivers, n_nodes):
    out = np.zeros((n_nodes, edges.shape[1]), np.float32)
    r = np.asarray(receivers).astype(np.int64)
    ok = (r >= 0) & (r < n_nodes)
    np.add.at(out, r[ok], np.asarray(edges, np.float32)[ok])
    return out


def kernel(edges, nodes, receivers):
    global LAST_RESULT

    edges = np.ascontiguousarray(edges, dtype=np.float32)
    n_nodes = nodes.shape[0]
    r = np.asarray(receivers).astype(np.int64)
    if (
        edges.shape != (N_EDGES, N_FEAT)
        or n_nodes != N_NODES
        or r.shape != (N_EDGES,)
        or ((r < 0) | (r >= N_NODES)).any()
        or os.environ.get("KERNEL_FORCE_NUMPY")
    ):
        return _numpy_segment_sum(edges, receivers, n_nodes)

    order = np.argsort(r, kind="stable")
    r_s = r[order]
    bounds = np.searchsorted(r_s, NPC * np.arange(N_CORES + 1))
    hi_all = edges.astype(np.float16)

    # ---- pass 1: per-core pseudo-node construction + sorted degree profiles
    per_core = []
    nb_max = 0
    for i in range(N_CORES):
        lo_b, hi_b = bounds[i], bounds[i + 1]
        idx = order[lo_b:hi_b]
        rr = (r_s[lo_b:hi_b] - NPC * i).astype(np.int64)
        d = np.bincount(rr, minlength=NPC)
        n_parts = np.maximum((d + K_CAP - 1) // K_CAP, 1)
        pseudo_base = _excl_cumsum(n_parts)
        n_pseudo = int(n_parts.sum())
        pseudo_orig = np.repeat(np.arange(NPC), n_parts)
        part_idx = np.arange(n_pseudo) - pseudo_base[pseudo_orig]
        pseudo_deg = np.minimum(d[pseudo_orig] - K_CAP * part_idx, K_CAP)
        sort_ord = np.argsort(-pseudo_deg, kind="stable")
        inv = np.empty(n_pseudo, np.int64)
        inv[sort_ord] = np.arange(n_pseudo)
        deg_sorted = pseudo_deg[sort_ord]
        per_core.append(
            (idx, rr, d, pseudo_base, inv, pseudo_orig, sort_ord, n_pseudo, deg_sorted)
        )
        nb_max = max(nb_max, (n_pseudo + BLK - 1) // BLK)

    # Static schedule: per-block tower height = ceil(block max degree / 2),
    # maxed over cores.
    k_all = np.zeros((N_CORES, nb_max), np.int64)
    for i in range(N_CORES):
        deg_sorted = per_core[i][8]
        nb_i = (len(deg_sorted) + BLK - 1) // BLK
        k_all[i, :nb_i] = (deg_sorted[0 : nb_i * BLK : BLK] + 1) // 2
    k_sched_arr = k_all.max(axis=0)
    nb = int(np.max(np.nonzero(k_sched_arr)[0])) + 1 if k_sched_arr.any() else 0
    if nb == 0:
        return np.zeros((N_NODES, N_FEAT), np.float32)
    k_sched = tuple(int(x) for x in k_sched_arr[:nb])
    c0 = np.concatenate([[0], np.cumsum(k_sched)]).astype(np.int64)
    c_total = int(c0[-1])

    nc = _build_nc(k_sched)

    # ---- pass 2: scatter edges into per-core token arrays
    ones2_np = np.zeros((128, 64), np.float16)
    ones2_np[np.arange(128), np.arange(128) // 2] = 1.0
    in_maps = []
    for i in range(N_CORES):
        idx, rr, d, pseudo_base, inv, _, _, _, _ = per_core[i]
        node_first = _excl_cumsum(d)
        rank = np.arange(len(rr)) - node_first[rr]
        pn = pseudo_base[rr] + rank // K_CAP
        rk = rank % K_CAP
        q = inv[pn]
        blk = q // BLK
        j = q % BLK
        part = 2 * j + (rk & 1)
        chunk = c0[blk] + (rk >> 1)
        tokens = np.zeros((128, c_total, 64), np.float16)
        tokens[part, chunk, :] = hi_all[idx]
        in_maps.append({"tokens": tokens, "ones2": ones2_np})

    from concourse.bass_utils import run_bass_kernel_spmd

    res = run_bass_kernel_spmd(nc, in_maps, core_ids=list(range(N_CORES)))
    LAST_RESULT = res

    # ---- unshard: pseudo-node sort_ord[q]'s sum lives at
    # dev[64*(blk&1) + j, blk>>1, :] with blk = q//64, j = q%64.
    full = np.zeros((N_NODES, N_FEAT), np.float32)
    for i in range(N_CORES):
        dev = res.results[i]["out"]  # [128, nbd, 64] f16
        _, _, _, _, _, pseudo_orig, sort_ord, n_pseudo, _ = per_core[i]
        m = min(n_pseudo, nb * BLK)  # trailing deg-0 pseudo-nodes may be trimmed
        q = np.arange(m)
        blk = q // BLK
        j = q % BLK
        vals = dev[64 * (blk & 1) + j, blk >> 1, :].astype(np.float32)
        block = full[i * NPC : (i + 1) * NPC]
        np.add.at(block, pseudo_orig[sort_ord[:m]], vals)

    return full


# revision 18
# speedup vs baseline: 1.0673x; 1.0673x over previous
"""Trainium2 Bass kernel: segment_sum of edge features into nodes (GNN
aggregation).

out[n, :] = sum of edges[e, :] over edges with receivers[e] == n, for
n in [0, 100000), edges [1000000, 64] fp32 — distributed over 8 NeuronCores.
Cores are value-sharded by receiver range (12500 nodes each, disjoint), so no
cross-core reduction is needed; the host concatenates the shards.

Device algorithm ("block-ones matmul tower fold", fp16 end to end):
  - Edge features ride as plain fp16 (end-to-end error ~5e-4 relative vs the
    2e-2 gate), halving input traffic vs an fp32-exact hi+lo split.
  - Host splits nodes with degree > 16 into pseudo-nodes of <= 16 edges,
    sorts pseudo-nodes by degree (desc), and packs 64 per block, 2 slots per
    node per chunk: pseudo-node j of block b puts its e-th edge row at
    tokens[2j + (e&1), c0[b] + (e>>1), :].  A block occupies
    K_b = ceil(max-degree-in-block / 2) <= 8 consecutive chunks ("towers");
    padding is ~3% (odd-degree slots + within-block degree spread).
  - ONE matmul per block folds the whole tower: lhsT = static block-ones
    [128, 64] (ones2[s, m] = 1 iff s//2 == m, so out row m sums slots 2m and
    2m+1), rhs = tok[:, c:c+K, :], and the out access pattern
    [[64 part], [0, K], [1, 64]] revisits the same 64 PSUM columns for every
    chunk — PSUM's per-element has_written accumulate sums the K chunks in
    hardware.  The 64-wide lhsT halves the per-matmul LDWEIGHTS cost (53 ns)
    vs a 128-wide identity, keeping the PE comfortably under the DMA stream.
  - Matmul out free iteration is ISA-capped at 512 elements, hence K <= 8 per
    instruction — guaranteed here since pseudo-degree <= 16.
  - Two blocks stack per 128-partition group (tile_position column tiling at
    partition 0/64); 16 blocks fill one 2KB PSUM bank; one ScalarE/VectorE
    copy (alternating) casts the bank to fp16 in SBUF.  Inputs stream on the
    Sync HWDGE ring in ~2 MB slabs; outputs ride the Scalar ring.  Output is
    exactly one 64-col fp16 row per pseudo-node (~1.7 MB/core).
  - Host adds pseudo-node rows back into node rows (np.add.at over ~13k rows)
    in f32.
  - Block heights K_b are measured from the actual data (elementwise max
    across the 8 cores' sorted degree profiles) and baked into the compiled
    program inside kernel(); all cores share one SPMD schedule.
"""

import os

import numpy as np

N_EDGES = 1_000_000
N_NODES = 100_000
N_FEAT = 64
N_CORES = 8
NPC = N_NODES // N_CORES  # 12500 nodes per core
K_CAP = 16  # max edges per pseudo-node -> tower height ceil(16/2) = 8 chunks
BLK = 64  # pseudo-nodes per block (two slots each)
BPB = 16  # blocks per PSUM bank (2 partition groups x 8 column slices)
CHUNK_BUDGET = 128  # chunks per input DMA slab (16 KB/partition, ~2.1 MB)

_NC_CACHE = {}
LAST_RESULT = None


def _excl_cumsum(a):
    s = np.zeros_like(a)
    np.cumsum(a[:-1], out=s[1:])
    return s


def _input_groups(k_sched):
    """One input DMA slab per PSUM bank group (16 blocks): the slab a bank
    group consumes is exactly one DMA, so the pipeline couples tightly and
    the schedule's sorted-descending heights give a naturally tapering tail."""
    nb = len(k_sched)
    return [[b, min(nb, b + BPB)] for b in range(0, nb, BPB)]


def _build_nc(k_sched):
    """Compile the SPMD program for a static tuple of block heights."""
    if k_sched in _NC_CACHE:
        return _NC_CACHE[k_sched]

    import concourse.bass as bass
    import concourse.tile as tile
    from concourse import bacc, mybir

    F16 = mybir.dt.float16
    F32 = mybir.dt.float32

    nb = len(k_sched)
    nbd = (nb + 1) // 2  # dram col-blocks (2 blocks stack per 128 partitions)
    c0 = np.concatenate([[0], np.cumsum(k_sched)]).astype(np.int64)
    c_total = int(c0[-1])
    igroups = _input_groups(k_sched)
    ng = len(igroups)
    gmax = max(int(c0[b1] - c0[b0]) for b0, b1 in igroups)

    nc = bacc.Bacc("TRN2", target_bir_lowering=False)
    tokens = nc.dram_tensor("tokens", [128, c_total, 64], F16, kind="ExternalInput")
    ones2 = nc.dram_tensor("ones2", [128, 64], F16, kind="ExternalInput")
    out = nc.dram_tensor("out", [128, nbd, 64], F16, kind="ExternalOutput")

    with tile.TileContext(nc) as tc:
        with (
            nc.allow_low_precision(reason="fp16 staging is intentional"),
            tc.tile_pool(name="const", bufs=1) as const,
            tc.tile_pool(name="tok", bufs=4) as tokp,
            tc.tile_pool(name="ps", bufs=4, space="PSUM") as psp,
            tc.tile_pool(name="stage", bufs=3) as stp,
        ):
            ones2_t = const.tile([128, 64], F16)
            nc.scalar.dma_start(ones2_t[:], ones2[:])

            def emit_slab(g, engine):
                b0, b1 = igroups[g]
                gc0, gcn = int(c0[b0]), int(c0[b1] - c0[b0])
                t = tokp.tile([128, gmax, 64], F16, tag="tok", name="tok")
                engine.dma_start(t[:, 0:gcn, :], tokens[:, gc0 : gc0 + gcn, :])
                return t, gc0

            slabs = {0: emit_slab(0, nc.sync)}
            if ng > 1:
                slabs[1] = emit_slab(1, nc.sync)

            for g in range(ng):
                if g + 2 < ng:
                    slabs[g + 2] = emit_slab(g + 2, nc.sync)
                tok, tok_c0 = slabs.pop(g)
                b0, b1 = igroups[g]
                ps = psp.tile([128, 512], F32, tag="ps")
                for b in range(b0, b1):
                    w = b - b0
                    k = k_sched[b]
                    assert 0 < k <= 8
                    cs = int(c0[b]) - tok_c0
                    prow = 64 * (w % 2)
                    slot = w // 2
                    pslice = ps[prow : prow + 64, slot * 64 : (slot + 1) * 64]
                    o = bass.AP(
                        pslice.tensor,
                        pslice.offset,
                        [list(pslice.ap[0]), [0, k], [1, 64]],
                    )
                    nc.tensor.matmul(
                        out=o,
                        lhsT=ones2_t[:],
                        rhs=tok[:, cs : cs + k, :],
                        start=True,
                        stop=True,
                    )
                ncols = ((b1 - 1 - b0) // 2 + 1) * 64
                stage = stp.tile([128, 512], F16, tag="stage")
                if g % 2:
                    nc.vector.tensor_copy(stage[:, 0:ncols], ps[:, 0:ncols])
                else:
                    nc.scalar.copy(stage[:, 0:ncols], ps[:, 0:ncols])
                nc.scalar.dma_start(
                    out[:, b0 // 2 : b0 // 2 + ncols // 64, :], stage[:, 0:ncols]
                )
    nc.compile()
    _NC_CACHE[k_sched] = nc
    return nc


def _numpy_segment_sum(edges, receivers, n_nodes):
    out = np.zeros((n_nodes, edges.shape[1]), np.float32)
    r = np.asarray(receivers).astype(np.int64)
    ok = (r >= 0) & (r < n_nodes)
    np.add.at(out, r[ok], np.asarray(edges, np.float32)[ok])
    return out


def kernel(edges, nodes, receivers):
    global LAST_RESULT

    edges = np.ascontiguousarray(edges, dtype=np.float32)
    n_nodes = nodes.shape[0]
    r = np.asarray(receivers).astype(np.int64)
    if (
        edges.shape != (N_EDGES, N_FEAT)
        or n_nodes != N_NODES
        or r.shape != (N_EDGES,)
        or ((r < 0) | (r >= N_NODES)).any()
        or os.environ.get("KERNEL_FORCE_NUMPY")
    ):
        return _numpy_segment_sum(edges, receivers, n_nodes)

    order = np.argsort(r, kind="stable")
    r_s = r[order]
    bounds = np.searchsorted(r_s, NPC * np.arange(N_CORES + 1))
    hi_all = edges.astype(np.float16)

    # ---- pass 1: per-core pseudo-node construction + sorted degree profiles
    per_core = []
    nb_max = 0
    for i in range(N_CORES):
        lo_b, hi_b = bounds[i], bounds[i + 1]
        idx = order[lo_b:hi_b]
        rr = (r_s[lo_b:hi_b] - NPC * i).astype(np.int64)
        d = np.bincount(rr, minlength=NPC)
        n_parts = np.maximum((d + K_CAP - 1) // K_CAP, 1)
        pseudo_base = _excl_cumsum(n_parts)
        n_pseudo = int(n_parts.sum())
        pseudo_orig = np.repeat(np.arange(NPC), n_parts)
        part_idx = np.arange(n_pseudo) - pseudo_base[pseudo_orig]
        pseudo_deg = np.minimum(d[pseudo_orig] - K_CAP * part_idx, K_CAP)
        sort_ord = np.argsort(-pseudo_deg, kind="stable")
        inv = np.empty(n_pseudo, np.int64)
        inv[sort_ord] = np.arange(n_pseudo)
        deg_sorted = pseudo_deg[sort_ord]
        per_core.append(
            (idx, rr, d, pseudo_base, inv, pseudo_orig, sort_ord, n_pseudo, deg_sorted)
        )
        nb_max = max(nb_max, (n_pseudo + BLK - 1) // BLK)

    # Static schedule: per-block tower height = ceil(block max degree / 2),
    # maxed over cores.
    k_all = np.zeros((N_CORES, nb_max), np.int64)
    for i in range(N_CORES):
        deg_sorted = per_core[i][8]
        nb_i = (len(deg_sorted) + BLK - 1) // BLK
        k_all[i, :nb_i] = (deg_sorted[0 : nb_i * BLK : BLK] + 1) // 2
    k_sched_arr = k_all.max(axis=0)
    nb = int(np.max(np.nonzero(k_sched_arr)[0])) + 1 if k_sched_arr.any() else 0
    if nb == 0:
        return np.zeros((N_NODES, N_FEAT), np.float32)
    k_sched = tuple(int(x) for x in k_sched_arr[:nb])
    c0 = np.concatenate([[0], np.cumsum(k_sched)]).astype(np.int64)
    c_total = int(c0[-1])

    nc = _build_nc(k_sched)

    # ---- pass 2: scatter edges into per-core token arrays
    ones2_np = np.zeros((128, 64), np.float16)
    ones2_np[np.arange(128), np.arange(128) // 2] = 1.0
    in_maps = []
    for i in range(N_CORES):
        idx, rr, d, pseudo_base, inv, _, _, _, _ = per_core[i]
        node_first = _excl_cumsum(d)
        rank = np.arange(len(rr)) - node_first[rr]
        pn = pseudo_base[rr] + rank // K_CAP
        rk = rank % K_CAP
        q = inv[pn]
        blk = q // BLK
        j = q % BLK
        part = 2 * j + (rk & 1)
        chunk = c0[blk] + (rk >> 1)
        tokens = np.zeros((128, c_total, 64), np.float16)
        tokens[part, chunk, :] = hi_all[idx]
        in_maps.append({"tokens": tokens, "ones2": ones2_np})

    from concourse.bass_utils import run_bass_kernel_spmd

    res = run_bass_kernel_spmd(nc, in_maps, core_ids=list(range(N_CORES)))
    LAST_RESULT = res

    # ---- unshard: pseudo-node sort_ord[q]'s sum lives at
    # dev[64*(blk&1) + j, blk>>1, :] with blk = q//64, j = q%64.
    full = np.zeros((N_NODES, N_FEAT), np.float32)
    for i in range(N_CORES):
        dev = res.results[i]["out"]  # [128, nbd, 64] f16
        _, _, _, _, _, pseudo_orig, sort_ord, n_pseudo, _ = per_core[i]
        m = min(n_pseudo, nb * BLK)  # trailing deg-0 pseudo-nodes may be trimmed
        q = np.arange(m)
        blk = q // BLK
        j = q % BLK
        vals = dev[64 * (blk & 1) + j, blk >> 1, :].astype(np.float32)
        block = full[i * NPC : (i + 1) * NPC]
        np.add.at(block, pseudo_orig[sort_ord[:m]], vals)

    return full


# revision 20
# speedup vs baseline: 1.1567x; 1.0838x over previous
"""Trainium2 Bass kernel: segment_sum of edge features into nodes (GNN
aggregation).

out[n, :] = sum of edges[e, :] over edges with receivers[e] == n, for
n in [0, 100000), edges [1000000, 64] fp32 — distributed over 8 NeuronCores.
Cores are value-sharded by receiver range (12500 nodes each, disjoint), so no
cross-core reduction is needed; the host concatenates the shards.

Device algorithm ("block-ones matmul tower fold", fp16 end to end):
  - Edge features ride as plain fp16 (end-to-end error ~5e-4 relative vs the
    2e-2 gate), halving input traffic vs an fp32-exact hi+lo split.
  - Host splits nodes with degree > 16 into pseudo-nodes of <= 16 edges,
    sorts pseudo-nodes by degree (desc), and packs 64 per block, 2 slots per
    node per chunk: pseudo-node j of block b puts its e-th edge row at
    tokens[2j + (e&1), c0[b] + (e>>1), :].  A block occupies
    K_b = ceil(max-degree-in-block / 2) <= 8 consecutive chunks ("towers");
    padding is ~3% (odd-degree slots + within-block degree spread).
  - ONE matmul per block folds the whole tower: lhsT = static block-ones
    [128, 64] (ones2[s, m] = 1 iff s//2 == m, so out row m sums slots 2m and
    2m+1), rhs = tok[:, c:c+K, :], and the out access pattern
    [[64 part], [0, K], [1, 64]] revisits the same 64 PSUM columns for every
    chunk — PSUM's per-element has_written accumulate sums the K chunks in
    hardware.  The 64-wide lhsT halves the per-matmul LDWEIGHTS cost (53 ns)
    vs a 128-wide identity, keeping the PE comfortably under the DMA stream.
  - Matmul out free iteration is ISA-capped at 512 elements, hence K <= 8 per
    instruction — guaranteed here since pseudo-degree <= 16.
  - Two blocks stack per 128-partition group (tile_position column tiling at
    partition 0/64); 16 blocks fill one 2KB PSUM bank; one ScalarE/VectorE
    copy (alternating) casts the bank to fp16 in SBUF.  Inputs stream on the
    Sync HWDGE ring in ~2 MB slabs; outputs ride the Scalar ring.  Output is
    exactly one 64-col fp16 row per pseudo-node (~1.7 MB/core).
  - Host adds pseudo-node rows back into node rows (np.add.at over ~13k rows)
    in f32.
  - Block heights K_b are measured from the actual data (elementwise max
    across the 8 cores' sorted degree profiles) and baked into the compiled
    program inside kernel(); all cores share one SPMD schedule.
"""

import os

import numpy as np

N_EDGES = 1_000_000
N_NODES = 100_000
N_FEAT = 64
N_CORES = 8
NPC = N_NODES // N_CORES  # 12500 nodes per core
K_CAP = 16  # max edges per pseudo-node -> tower height ceil(16/2) = 8 chunks
BLK = 64  # pseudo-nodes per block (two slots each)
BPB = 16  # blocks per PSUM bank (2 partition groups x 8 column slices)
CHUNK_BUDGET = 128  # chunks per input DMA slab (16 KB/partition, ~2.1 MB)

_NC_CACHE = {}
LAST_RESULT = None


def _excl_cumsum(a):
    s = np.zeros_like(a)
    np.cumsum(a[:-1], out=s[1:])
    return s


def _input_groups(k_sched):
    """One input DMA slab per PSUM bank group (16 blocks): the slab a bank
    group consumes is exactly one DMA, so the pipeline couples tightly and
    the schedule's sorted-descending heights give a naturally tapering tail."""
    nb = len(k_sched)
    return [[b, min(nb, b + BPB)] for b in range(0, nb, BPB)]


def _build_nc(k_sched):
    """Compile the SPMD program for a static tuple of block heights."""
    if k_sched in _NC_CACHE:
        return _NC_CACHE[k_sched]

    import concourse.bass as bass
    import concourse.tile as tile
    from concourse import bacc, mybir

    F16 = mybir.dt.float16
    F32 = mybir.dt.float32

    nb = len(k_sched)
    nbd = (nb + 1) // 2  # dram col-blocks (2 blocks stack per 128 partitions)
    c0 = np.concatenate([[0], np.cumsum(k_sched)]).astype(np.int64)
    c_total = int(c0[-1])
    igroups = _input_groups(k_sched)
    ng = len(igroups)
    gmax = max(int(c0[b1] - c0[b0]) for b0, b1 in igroups)

    nc = bacc.Bacc("TRN2", target_bir_lowering=False)
    tokens = nc.dram_tensor("tokens", [128, c_total, 64], F16, kind="ExternalInput")
    ones2 = nc.dram_tensor("ones2", [128, 64], F16, kind="ExternalInput")
    out = nc.dram_tensor("out", [128, nbd, 64], F16, kind="ExternalOutput")

    with tile.TileContext(nc) as tc:
        with (
            nc.allow_low_precision(reason="fp16 staging is intentional"),
            tc.tile_pool(name="const", bufs=1) as const,
            tc.tile_pool(name="tok", bufs=5) as tokp,
            tc.tile_pool(name="ps", bufs=6, space="PSUM") as psp,
            tc.tile_pool(name="stage", bufs=3) as stp,
        ):
            ones2_t = const.tile([128, 64], F16)
            nc.scalar.dma_start(ones2_t[:], ones2[:])

            def emit_slab(g, engine):
                b0, b1 = igroups[g]
                gc0, gcn = int(c0[b0]), int(c0[b1] - c0[b0])
                t = tokp.tile([128, gmax, 64], F16, tag="tok", name="tok")
                engine.dma_start(t[:, 0:gcn, :], tokens[:, gc0 : gc0 + gcn, :])
                return t, gc0

            slabs = {0: emit_slab(0, nc.sync)}
            if ng > 1:
                slabs[1] = emit_slab(1, nc.sync)

            for g in range(ng):
                if g + 2 < ng:
                    slabs[g + 2] = emit_slab(g + 2, nc.sync)
                tok, tok_c0 = slabs.pop(g)
                b0, b1 = igroups[g]
                ps = psp.tile([128, 512], F32, tag="ps")
                consumed = set()
                for b in range(b0, b1):
                    if b in consumed:
                        continue
                    w = b - b0
                    k = k_sched[b]
                    assert 0 < k <= 8
                    cs = int(c0[b]) - tok_c0
                    prow = 64 * (w % 2)
                    slot = w // 2
                    # Merge (b, b+2) — same partition group, adjacent PSUM
                    # slots — into one matmul when both towers have equal
                    # K <= 4 (out iteration 2*K*64 <= 512).
                    if (
                        k <= 4
                        and b + 2 < b1
                        and b + 2 not in consumed
                        and k_sched[b + 2] == k
                    ):
                        cs2 = int(c0[b + 2]) - tok_c0
                        pslice = ps[prow : prow + 64, slot * 64 : (slot + 2) * 64]
                        o = bass.AP(
                            pslice.tensor,
                            pslice.offset,
                            [list(pslice.ap[0]), [64, 2], [0, k], [1, 64]],
                        )
                        rhs = bass.AP(
                            tok.tensor,
                            tok.offset + cs * 64,
                            [list(tok.ap[0]), [(cs2 - cs) * 64, 2], [64, k], [1, 64]],
                        )
                        nc.tensor.matmul(
                            out=o, lhsT=ones2_t[:], rhs=rhs, start=True, stop=True
                        )
                        consumed.add(b + 2)
                        continue
                    pslice = ps[prow : prow + 64, slot * 64 : (slot + 1) * 64]
                    o = bass.AP(
                        pslice.tensor,
                        pslice.offset,
                        [list(pslice.ap[0]), [0, k], [1, 64]],
                    )
                    nc.tensor.matmul(
                        out=o,
                        lhsT=ones2_t[:],
                        rhs=tok[:, cs : cs + k, :],
                        start=True,
                        stop=True,
                    )
                ncols = ((b1 - 1 - b0) // 2 + 1) * 64
                stage = stp.tile([128, 512], F16, tag="stage")
                if g % 2:
                    nc.vector.tensor_copy(stage[:, 0:ncols], ps[:, 0:ncols])
                else:
                    nc.scalar.copy(stage[:, 0:ncols], ps[:, 0:ncols])
                nc.scalar.dma_start(
                    out[:, b0 // 2 : b0 // 2 + ncols // 64, :], stage[:, 0:ncols]
                )
    nc.compile()
    _NC_CACHE[k_sched] = nc
    return nc


def _numpy_segment_sum(edges, receivers, n_nodes):
    out = np.zeros((n_nodes, edges.shape[1]), np.float32)
    r = np.asarray(receivers).astype(np.int64)
    ok = (r >= 0) & (r < n_nodes)
    np.add.at(out, r[ok], np.asarray(edges, np.float32)[ok])
    return out


def kernel(edges, nodes, receivers):
    global LAST_RESULT

    edges = np.ascontiguousarray(edges, dtype=np.float32)
    n_nodes = nodes.shape[0]
    r = np.asarray(receivers).astype(np.int64)
    if (
        edges.shape != (N_EDGES, N_FEAT)
        or n_nodes != N_NODES
        or r.shape != (N_EDGES,)
        or ((r < 0) | (r >= N_NODES)).any()
        or os.environ.get("KERNEL_FORCE_NUMPY")
    ):
        return _numpy_segment_sum(edges, receivers, n_nodes)

    order = np.argsort(r, kind="stable")
    r_s = r[order]
    bounds = np.searchsorted(r_s, NPC * np.arange(N_CORES + 1))
    hi_all = edges.astype(np.float16)

    # ---- pass 1: per-core pseudo-node construction + sorted degree profiles
    per_core = []
    nb_max = 0
    for i in range(N_CORES):
        lo_b, hi_b = bounds[i], bounds[i + 1]
        idx = order[lo_b:hi_b]
        rr = (r_s[lo_b:hi_b] - NPC * i).astype(np.int64)
        d = np.bincount(rr, minlength=NPC)
        n_parts = np.maximum((d + K_CAP - 1) // K_CAP, 1)
        pseudo_base = _excl_cumsum(n_parts)
        n_pseudo = int(n_parts.sum())
        pseudo_orig = np.repeat(np.arange(NPC), n_parts)
        part_idx = np.arange(n_pseudo) - pseudo_base[pseudo_orig]
        pseudo_deg = np.minimum(d[pseudo_orig] - K_CAP * part_idx, K_CAP)
        sort_ord = np.argsort(-pseudo_deg, kind="stable")
        inv = np.empty(n_pseudo, np.int64)
        inv[sort_ord] = np.arange(n_pseudo)
        deg_sorted = pseudo_deg[sort_ord]
        per_core.append(
            (idx, rr, d, pseudo_base, inv, pseudo_orig, sort_ord, n_pseudo, deg_sorted)
        )
        nb_max = max(nb_max, (n_pseudo + BLK - 1) // BLK)

    # Static schedule: per-block tower height = ceil(block max degree / 2),
    # maxed over cores.
    k_all = np.zeros((N_CORES, nb_max), np.int64)
    for i in range(N_CORES):
        deg_sorted = per_core[i][8]
        nb_i = (len(deg_sorted) + BLK - 1) // BLK
        k_all[i, :nb_i] = (deg_sorted[0 : nb_i * BLK : BLK] + 1) // 2
    k_sched_arr = k_all.max(axis=0)
    nb = int(np.max(np.nonzero(k_sched_arr)[0])) + 1 if k_sched_arr.any() else 0
    if nb == 0:
        return np.zeros((N_NODES, N_FEAT), np.float32)
    k_sched = tuple(int(x) for x in k_sched_arr[:nb])
    c0 = np.concatenate([[0], np.cumsum(k_sched)]).astype(np.int64)
    c_total = int(c0[-1])

    nc = _build_nc(k_sched)

    # ---- pass 2: scatter edges into per-core token arrays
    ones2_np = np.zeros((128, 64), np.float16)
    ones2_np[np.arange(128), np.arange(128) // 2] = 1.0
    in_maps = []
    for i in range(N_CORES):
        idx, rr, d, pseudo_base, inv, _, _, _, _ = per_core[i]
        node_first = _excl_cumsum(d)
        rank = np.arange(len(rr)) - node_first[rr]
        pn = pseudo_base[rr] + rank // K_CAP
        rk = rank % K_CAP
        q = inv[pn]
        blk = q // BLK
        j = q % BLK
        part = 2 * j + (rk & 1)
        chunk = c0[blk] + (rk >> 1)
        tokens = np.zeros((128, c_total, 64), np.float16)
        tokens[part, chunk, :] = hi_all[idx]
        in_maps.append({"tokens": tokens, "ones2": ones2_np})

    from concourse.bass_utils import run_bass_kernel_spmd

    res = run_bass_kernel_spmd(nc, in_maps, core_ids=list(range(N_CORES)))
    LAST_RESULT = res

    # ---- unshard: pseudo-node sort_ord[q]'s sum lives at
    # dev[64*(blk&1) + j, blk>>1, :] with blk = q//64, j = q%64.
    full = np.zeros((N_NODES, N_FEAT), np.float32)
    for i in range(N_CORES):
        dev = res.results[i]["out"]  # [128, nbd, 64] f16
        _, _, _, _, _, pseudo_orig, sort_ord, n_pseudo, _ = per_core[i]
        m = min(n_pseudo, nb * BLK)  # trailing deg-0 pseudo-nodes may be trimmed
        q = np.arange(m)
        blk = q // BLK
        j = q % BLK
        vals = dev[64 * (blk & 1) + j, blk >> 1, :].astype(np.float32)
        block = full[i * NPC : (i + 1) * NPC]
        np.add.at(block, pseudo_orig[sort_ord[:m]], vals)

    return full
